# revision 39
# baseline (speedup 1.0000x reference)
"""BlockTucker kernel for TRN2, 8 NeuronCores, data-parallel over batch.

Model (per reference):
    h0 = (x0 @ W0.T + b0).reshape(B, C, S)          B=8192 DIN=2048 MM=1600
    h1 = (x1 @ W1.T + b1).reshape(B, C, S)          C=20 chunks, S=80
    z[b,c,q] = sum_{s,t} h0[b,c,s] Wb[c,q,s,t] h1[b,c,t] + bb[c,q]
    z = signed_sqrt(z); z = z / max(||z||_chunk, eps); out = z @ Wout.T + bout

Per-core dataflow (BL = 1024 rows/core, all params replicated, all bf16):
  stage A (PE): h[b, m] = xT_a.T @ WT_a per batch tile (bias folded as a
      K=1 ones-row pass); evac into a 128-blocked scratch; a DMA-transpose
      produces chunk-aligned hsb[s, bt, c, b].
  middle, per chunk c / (q,t)-tile kt (50 of 128) / half-unit un (4 bt):
      mm1 (PE): y2[j=(q,t), b] = WbT[c][:, kt].T @ h0sb[:, bsl, c]  (K=80)
      gate to bf16 SBUF, path per unit (D: DVE direct from PSUM;
          A: ACT evac + DVE 2x gate; G: ACT evac + Pool/DVE split gate):
          g = y2 * h1rot[(q,t)%80-rotated]
      mm2 (PE): z[b, q] += g[:, b-slice].T @ SEL  (tiny selection matmuls
          accumulating the t-reduction in PSUM at ~2 cols each)
  tail (+bb, signed sqrt, chunk-normalize) in [b, m] layout; DMA-transpose
      zn -> znT; out-proj (PE) with bias as an extra znT ones-row.
"""

import numpy as np

BL = 1024          # batch rows per core
DIN = 2048
MM = 1600
C, S = 20, 80
OUT = 3000
NCORES = 8
EPS = 1e-12
NBT = BL // 128    # 8 batch tiles
NKQ = C * S * S // (C * 128)  # 50 (q,t)-tiles of 128 per chunk
OFFS = [0, 48, 16, 64, 32]    # (128*kt) % 80 for kt % 5
# per-gate-unit path assignment weights (any total; spread evenly):
#   D = DVE gate direct from PSUM      (~660 ns/half-unit on DVE)
#   A = ACT evac bf16 + DVE gate 2x    (~805 ns ACT + ~400 ns DVE)
#   G = ACT evac bf16 + Pool gate      (~805 ns ACT + ~1150 ns Pool)
# (Pool cannot access PSUM on TRN2 -- walrus rejects it -- so every Pool
#  gate needs an ACT evac first.)
RATIOS = {"D": 340, "A": 330, "G": 330}
PATTERN_OVERRIDE = "DAG"  # non-empty: repeat this path string instead of RATIOS
GSPLIT = 3             # in a G unit, Pool gates bt < GSPLIT, DVE the rest
ZERO_ENGINE = "pe"     # which engine zeroes psZ: "pool" | "dve" | "pe"
ZSB_ENGINE = "act"     # which engine drains psZ to zsb: "act" | "pool"
FLUSH_LAG = 8          # pend-queue depth (units) before mm2 flush
UBT = 8                # batch-tiles per gate unit
PSY_BUFS = 3
PSZ_BUFS = 1
PSA_BUFS = 2
GP_BUFS = 8
WAIT_CAP = 1
EVP_BUFS = 4

_CACHE = {}


def _gate_pattern(n):
    """Evenly interleaved path assignment (Bresenham spread)."""
    if PATTERN_OVERRIDE:
        return (PATTERN_OVERRIDE * (n // len(PATTERN_OVERRIDE) + 1))[:n]
    tot = sum(RATIOS.values())
    emitted = {k: 0 for k in RATIOS}
    out = []
    for u in range(n):
        best, berr = None, None
        for k, w in RATIOS.items():
            err = w * (u + 1) / tot - emitted[k]
            if berr is None or err > berr:
                best, berr = k, err
        emitted[best] += 1
        out.append(best)
    return "".join(out)


def _build():
    import concourse.bass as bass
    import concourse.mybir as mybir
    import concourse.tile as tile

    f32 = mybir.dt.float32
    bf16 = mybir.dt.bfloat16
    AF = mybir.ActivationFunctionType
    ALU = mybir.AluOpType
    AX = mybir.AxisListType

    nc = bass.Bass()

    x0a = nc.declare_dram_parameter("x0a", [NBT, 128, 16, 128], bf16, isOutput=False)
    x1a = nc.declare_dram_parameter("x1a", [NBT, 128, 16, 128], bf16, isOutput=False)
    w0a = nc.declare_dram_parameter("w0a", [128, 16, MM], bf16, isOutput=False)
    w1a = nc.declare_dram_parameter("w1a", [128, 16, MM], bf16, isOutput=False)
    b01 = nc.declare_dram_parameter("b01", [1, 2, MM], bf16, isOutput=False)
    wbt = nc.declare_dram_parameter("wbt", [C, S, S * S], bf16, isOutput=False)
    selp = nc.declare_dram_parameter("selp", [128, 5, 4], bf16, isOutput=False)
    bbr = nc.declare_dram_parameter("bbr", [MM], bf16, isOutput=False)
    wot = nc.declare_dram_parameter("wot", [128, 13, OUT], bf16, isOutput=False)
    out = nc.declare_dram_parameter("out", [BL, OUT], f32, isOutput=True)

    with tile.TileContext(nc) as tc:
        from contextlib import ExitStack

        with ExitStack() as top:
            const = top.enter_context(tc.tile_pool(name="const", bufs=1))
            sels = const.tile([128, 5, 4], bf16)
            nc.sync.dma_start(out=sels[:], in_=selp[:])
            zer = const.tile([128, 512], bf16)
            nc.vector.memset(zer[:], 0.0)

            zsb_pool = top.enter_context(tc.tile_pool(name="zsbp", bufs=1))

            with ExitStack() as hes:
                hsb_pool = hes.enter_context(tc.tile_pool(name="hsbp", bufs=1))
                # chunk-aligned activations: [s, bt, c(128-blocked), b]
                h0sb = hsb_pool.tile([S, NBT, C, 128], bf16)
                h1sb = hsb_pool.tile([S, NBT, C, 128], bf16)

                # ================= stage A =================
                with ExitStack() as aes:
                    xwp = aes.enter_context(tc.tile_pool(name="xwp", bufs=4))
                    xbtp = aes.enter_context(tc.tile_pool(name="xbtp", bufs=2))
                    psA = aes.enter_context(
                        tc.tile_pool(name="psA", bufs=PSA_BUFS, space="PSUM")
                    )
                    scrp = aes.enter_context(tc.tile_pool(name="scrp", bufs=2))
                    cA = aes.enter_context(tc.tile_pool(name="cA", bufs=1))
                    b01s = cA.tile([1, 2, MM], bf16)
                    nc.sync.dma_start(out=b01s[:], in_=b01[:])
                    ones1 = cA.tile([1, 128], bf16)
                    nc.vector.memset(ones1[:], 1.0)

                    for proj, (x_d, w_d, hsb) in enumerate(
                        ((x0a, w0a, h0sb), (x1a, w1a, h1sb))
                    ):
                        if proj == 0:
                            xb0 = xbtp.tile([128, 16, 128], bf16, tag="xb")
                            nc.sync.dma_start(out=xb0[:], in_=x_d[0])
                        wah = []
                        for hf in range(4):
                            wt = xwp.tile([128, 16, 400], bf16, tag="wah")
                            nc.sync.dma_start(
                                out=wt[:], in_=w_d[:, :, hf * 400 : (hf + 1) * 400]
                            )
                            wah.append(wt)
                        for bt in range(NBT):
                            if proj == 0 and bt == 0:
                                xb = xb0
                            else:
                                xb = xbtp.tile([128, 16, 128], bf16, tag="xb")
                                nc.sync.dma_start(out=xb[:], in_=x_d[bt])
                            scr = scrp.tile([128, C * 128], bf16, tag="scr")
                            scrv = scr[:].rearrange("p (c s) -> p c s", s=128)
                            for qr in range(4):
                                msl = slice(qr * 400, (qr + 1) * 400)
                                ps = psA.tile([128, 512], f32, tag="ps")
                                for kt in range(16):
                                    nc.tensor.matmul(
                                        ps[:, :400],
                                        lhsT=xb[:, kt, :],
                                        rhs=wah[qr][:, kt, :],
                                        start=(kt == 0),
                                        stop=False,
                                    )
                                nc.tensor.matmul(
                                    ps[:, :400],
                                    lhsT=ones1[:],
                                    rhs=b01s[:, proj, msl],
                                    start=False,
                                    stop=True,
                                )
                                # 128-blocked scratch: col c*128+s = m c*80+s
                                nc.scalar.copy(
                                    scrv[:, qr * 5 : qr * 5 + 5, :S],
                                    ps[:, :400].rearrange(
                                        "p (c s) -> p c s", s=S
                                    ),
                                )
                            nc.sync.dma_start_transpose(
                                out=hsb[:, bt], in_=scr[:]
                            )

                # ================= middle =================
                zsb = zsb_pool.tile([128, NBT, C, S], bf16)
                with ExitStack() as mes:
                    wbp = mes.enter_context(tc.tile_pool(name="wbp", bufs=2))
                    rotp = mes.enter_context(tc.tile_pool(name="rotp", bufs=2))
                    psY = mes.enter_context(
                        tc.tile_pool(name="psY", bufs=PSY_BUFS, space="PSUM")
                    )
                    psZ = mes.enter_context(
                        tc.tile_pool(name="psZ", bufs=PSZ_BUFS, space="PSUM")
                    )
                    evp = mes.enter_context(
                        tc.tile_pool(name="evp", bufs=EVP_BUFS)
                    )
                    gp = mes.enter_context(tc.tile_pool(name="gp", bufs=GP_BUFS))

                    def prep(c):
                        wb = wbp.tile([S, S * S], bf16, tag="wb")
                        nc.sync.dma_start(out=wb[:], in_=wbt[c])
                        h1rot = rotp.tile([128, 5, NBT, 128], bf16, tag="rot")
                        for r in range(5):
                            o = OFFS[r]
                            j = 0
                            while j < 128:
                                t0 = (o + j) % S
                                n = min(S - t0, 128 - j)
                                nc.sync.dma_start(
                                    out=h1rot[j : j + n, r],
                                    in_=h1sb[t0 : t0 + n, :, c, :],
                                )
                                j += n
                        return wb, h1rot

                    cur = prep(0)
                    pend = []
                    nun = NBT // UBT  # gate units per kt
                    pattern = _gate_pattern(C * NKQ * nun)

                    def flush_one():
                        kt_, un_, g_, zp_, cc_ = pend.pop(0)
                        if kt_ == 0 and un_ == 0:
                            # lazy PSUM zeroing; emitted after the previous
                            # chunk's evac so the WAR dep cannot deadlock
                            if ZERO_ENGINE == "pool":
                                nc.gpsimd.memset(zp_[:], 0.0)
                            elif ZERO_ENGINE == "dve":
                                nc.vector.memset(zp_[:], 0.0)
                            else:
                                zpf = zp_[:].rearrange("p bt b -> p (bt b)")
                                for half in range(2):
                                    nc.tensor.matmul(
                                        zpf[:, half * 512 : half * 512 + 512],
                                        lhsT=zer[:, :128], rhs=zer[:],
                                        start=True, stop=False,
                                        skip_group_check=True,
                                    )
                        emit_mm2(kt_, un_, g_, zp_, cc_)
                        if kt_ == NKQ - 1 and un_ == nun - 1:
                            if ZSB_ENGINE == "pool":
                                nc.gpsimd.tensor_scalar_add(
                                    out=zsb[:, :, cc_, :],
                                    in0=zp_[:, :, :S],
                                    scalar1=0.0,
                                )
                            else:
                                nc.scalar.copy(zsb[:, :, cc_, :], zp_[:, :, :S])

                    for c in range(C):
                        wb, h1rot = cur

                        zps = psZ.tile([128, NBT, 128], f32, tag="zps")

                        h0c = h0sb[:, :, c, :]

                        def emit_mm2(kt, un, g, zp, cc):
                            r = kt % 5
                            q_lo = (128 * kt) // S
                            q_hi = (128 * kt + 127) // S
                            w = q_hi - q_lo + 1
                            last = kt == NKQ - 1
                            for bi in range(UBT):
                                bt = un * UBT + bi
                                nc.tensor.matmul(
                                    zp[:, bt, q_lo : q_lo + w],
                                    lhsT=g[:, bi, :], rhs=sels[:, r, :w],
                                    start=False, stop=last,
                                    skip_group_check=True,
                                )

                        for kt in range(NKQ):
                            if kt == 10 and c + 1 < C:
                                cur = prep(c + 1)
                            r = kt % 5
                            for un in range(nun):
                                bsl = slice(un * UBT, (un + 1) * UBT)
                                path = pattern[(c * NKQ + kt) * nun + un]
                                y2 = psY.tile([128, UBT, 128], f32, tag="y2")
                                for bh in range(0, UBT, 4):
                                    bw = min(4, UBT - bh)
                                    nc.tensor.matmul(
                                        y2[:, bh : bh + bw, :],
                                        lhsT=wb[:, kt * 128 : (kt + 1) * 128],
                                        rhs=h0c[
                                            :,
                                            un * UBT + bh : un * UBT + bh + bw,
                                            :,
                                        ],
                                        start=True,
                                        stop=True,
                                    )
                                g = gp.tile([128, UBT, 128], bf16, tag="g")
                                h1r = h1rot[:, r, bsl]
                                if path == "D":
                                    nc.vector.tensor_tensor(
                                        out=g[:], in0=y2[:], in1=h1r,
                                        op=ALU.mult,
                                    )
                                elif path == "A":
                                    y2b = evp.tile(
                                        [128, UBT, 128], bf16, tag="y2b"
                                    )
                                    nc.scalar.copy(y2b[:], y2[:])
                                    nc.vector.tensor_tensor(
                                        out=g[:], in0=y2b[:], in1=h1r,
                                        op=ALU.mult,
                                    )
                                else:  # G: ACT evac; Pool/DVE split gate
                                    y2b = evp.tile(
                                        [128, UBT, 128], bf16, tag="y2b"
                                    )
                                    nc.scalar.copy(y2b[:], y2[:])
                                    gs = min(GSPLIT, UBT)
                                    nc.gpsimd.tensor_tensor(
                                        out=g[:, :gs],
                                        in0=y2b[:, :gs],
                                        in1=h1r[:, :gs],
                                        op=ALU.mult,
                                    )
                                    if gs < UBT:
                                        nc.vector.tensor_tensor(
                                            out=g[:, gs:],
                                            in0=y2b[:, gs:],
                                            in1=h1r[:, gs:],
                                            op=ALU.mult,
                                        )
                                pend.append((kt, un, g, zps, c))
                                if len(pend) >= FLUSH_LAG:
                                    flush_one()

                    while pend:
                        flush_one()

            # ============ tail + out-proj ============
            with ExitStack() as oes:
                znp = oes.enter_context(tc.tile_pool(name="znp", bufs=2))
                znTp = oes.enter_context(tc.tile_pool(name="znTp", bufs=1))
                wop = oes.enter_context(tc.tile_pool(name="wop", bufs=1))
                psO = oes.enter_context(
                    tc.tile_pool(name="psO", bufs=2, space="PSUM")
                )
                osbp = oes.enter_context(tc.tile_pool(name="osbp", bufs=3))
                tp = oes.enter_context(tc.tile_pool(name="tp", bufs=1))
                sp = oes.enter_context(tc.tile_pool(name="sp", bufs=2))

                bbrep = wop.tile([128, MM], bf16)
                nc.sync.dma_start(
                    out=bbrep[:],
                    in_=bbr[:].unsqueeze(0).broadcast_to([128, MM]),
                )
                woT = wop.tile([128, 13, OUT], bf16)
                for og in range(6):
                    osl = slice(og * 500, (og + 1) * 500)
                    nc.sync.dma_start(out=woT[:, :, osl], in_=wot[:, :, osl])
                znT = znTp.tile([128, NBT, 13, 128], bf16)

                for bt in range(NBT):
                    zt = zsb[:, bt].rearrange("p c q -> p (c q)")
                    u = tp.tile([128, MM], bf16, tag="u")
                    nc.vector.tensor_tensor(
                        out=u[:], in0=zt, in1=bbrep[:], op=ALU.add
                    )
                    sg = tp.tile([128, MM], bf16, tag="sg")
                    nc.scalar.activation(sg[:], u[:], AF.Sign)
                    ab = tp.tile([128, MM], bf16, tag="ab")
                    nc.scalar.activation(ab[:], u[:], AF.Abs)
                    sq = tp.tile([128, MM], bf16, tag="sq")
                    nc.scalar.activation(sq[:], ab[:], AF.Sqrt)
                    ss = tp.tile([128, MM], bf16, tag="ss")
                    nc.vector.tensor_tensor(
                        out=ss[:], in0=sg[:], in1=sq[:], op=ALU.mult
                    )
                    # ||chunk||^2 = sum |u| per chunk
                    nsq = sp.tile([128, C], f32, tag="nsq")
                    nc.vector.tensor_reduce(
                        out=nsq[:],
                        in_=u[:].rearrange("p (c q) -> p c q", q=S),
                        axis=AX.X, op=ALU.add, apply_absolute_value=True,
                    )
                    nrm = sp.tile([128, C], f32, tag="nrm")
                    nc.scalar.activation(nrm[:], nsq[:], AF.Sqrt)
                    nrx = sp.tile([128, C], f32, tag="nrx")
                    nc.vector.tensor_scalar_max(out=nrx[:], in0=nrm[:], scalar1=EPS)
                    inv = sp.tile([128, C], f32, tag="inv")
                    nc.vector.reciprocal(inv[:], nrx[:])
                    zn2 = znp.tile([128, 13 * 128], bf16, tag="zn2")
                    nc.vector.tensor_tensor(
                        out=zn2[:, :MM].rearrange("p (c q) -> p c q", q=S),
                        in0=ss[:].rearrange("p (c q) -> p c q", q=S),
                        in1=inv[:].unsqueeze(2).broadcast_to([128, C, S]),
                        op=ALU.mult,
                    )
                    nc.vector.memset(zn2[:, MM:], 1.0)  # bias ones-row m=1600
                    nc.sync.dma_start_transpose(out=znT[:, bt], in_=zn2[:])

                    for og in range(6):
                        osl = slice(og * 500, (og + 1) * 500)
                        ps = psO.tile([128, 512], f32, tag="po")
                        for kt in range(13):
                            K = 128 if kt < 12 else 65
                            nc.tensor.matmul(
                                ps[:, :500],
                                lhsT=znT[:K, bt, kt, :],
                                rhs=woT[:K, kt, osl],
                                start=(kt == 0),
                                stop=(kt == 12),
                            )
                        ob = osbp.tile([128, 500], f32, tag="ob")
                        nc.scalar.copy(ob[:], ps[:, :500])
                        nc.sync.dma_start(
                            out=out[bt * 128 : (bt + 1) * 128, osl], in_=ob[:]
                        )

    _split_excess_waits(nc, cap=4)
    return nc


def _split_excess_waits(nc, cap=4):
    """Walrus rejects instructions with too many sync waits. Move excess
    waits onto NoOps spliced just before the instruction on the same engine
    queue (the sequencer executes them in order, so semantics are identical).
    """
    import concourse.mybir as mybir
    import bass_rust

    n = 0
    for f in nc.m.functions:
        for blk in f.blocks:
            out = []
            changed = False
            for inst in blk.instructions:
                si = getattr(inst, "sync_info", None)
                waits = list(si.on_wait) if si is not None and si.on_wait else []
                if inst.opcode == "EventSemaphore":
                    icap = 2
                elif inst.opcode in (
                    "Matmult",
                    "Ldweights",
                    "DMACopy",
                    "DmaTransposeAnt",
                    "TriggerDma",
                    "Memset",
                ):
                    # hw-decoded / DMA formats accept a single sync wait
                    icap = 1
                else:
                    icap = WAIT_CAP
                if len(waits) > icap:
                    excess, keep = waits[:-icap], waits[-icap:]
                    for w in excess:
                        nop = mybir.InstNoOp(
                            name=f"{inst.name}-wsplit{n}", ins=[], outs=[]
                        )
                        n += 1
                        nop.engine = inst.engine
                        nop.sync_info = bass_rust.SyncInfo(
                            on_wait=[w], on_update=[]
                        )
                        out.append(nop)
                    inst.sync_info = bass_rust.SyncInfo(
                        on_wait=keep, on_update=list(si.on_update or [])
                    )
                    changed = True
                out.append(inst)
            if changed:
                blk.instructions = out
    return nc


def _get_nc():
    if "nc" not in _CACHE:
        _CACHE["nc"] = _build()
    return _CACHE["nc"]


def _prep_core(inputs, lo, hi, bf):
    """Host-side layout prep for one core's batch slice [lo, hi)."""
    x0 = inputs["x0"][lo:hi]
    x1 = inputs["x1"][lo:hi]

    def xtiles(x):
        # [NBT, 128, 16, 128]: (bt, p, kt, b) = x[bt*128+b, kt*128+p]
        xt = x.reshape(NBT, 128, 16, 128)  # (bt, b, kt, p)
        return np.ascontiguousarray(xt.transpose(0, 3, 2, 1)).astype(bf)

    m = dict(_CACHE["shared"])
    m["x0a"] = xtiles(x0)
    m["x1a"] = xtiles(x1)
    return m


def _prep_shared(inputs, bf):
    def wtiles(w):
        # [128, 16, MM]: (p, kt, m) = W[m, kt*128+p]
        wt = np.ascontiguousarray(w.T).reshape(16, 128, MM)
        return np.ascontiguousarray(wt.transpose(1, 0, 2)).astype(bf)

    sel = np.zeros((128, 5, 4), np.float32)
    for r, o in enumerate(OFFS):
        for j in range(128):
            sel[j, r, (o + j) // S] = 1.0

    wo = np.zeros((13 * 128, OUT), np.float32)
    wo[:MM] = inputs["Wout"].T
    wo[MM] = inputs["bout"]
    wo = wo.reshape(13, 128, OUT).transpose(1, 0, 2)

    return {
        "w0a": wtiles(inputs["W0"]),
        "w1a": wtiles(inputs["W1"]),
        "b01": np.stack([inputs["b0"], inputs["b1"]])[None].astype(bf),
        "wbt": np.ascontiguousarray(
            inputs["Wb"].transpose(0, 2, 1, 3)
        ).reshape(C, S, S * S).astype(bf),
        "selp": sel.astype(bf),
        "bbr": inputs["bb"].reshape(MM).astype(bf),
        "wot": np.ascontiguousarray(wo).astype(bf),
    }


def kernel(**inputs):
    import ml_dtypes
    from concourse.bass_utils import run_bass_kernel_spmd

    bf = ml_dtypes.bfloat16
    nc = _get_nc()
    full = {k: np.asarray(v, dtype=np.float32) for k, v in inputs.items()}
    _CACHE["shared"] = _prep_shared(full, bf)
    rows = full["x0"].shape[0] // NCORES
    in_maps = [
        _prep_core(full, i * rows, (i + 1) * rows, bf) for i in range(NCORES)
    ]
    res = run_bass_kernel_spmd(nc, in_maps, list(range(NCORES)))
    return np.concatenate([res.results[i]["out"] for i in range(NCORES)], axis=0)
